# revision 15
# baseline (speedup 1.0000x reference)
"""Trainium2 Bass kernel for nn_CurvatureAwareLoss.

Strategy (8 NeuronCores, symmetric data-parallel over batch rows):
  host:   sort rows by label. Core c owns rows [512c, 512c+512) and the
          upper-triangle-ish band of columns: global 512-col chunks
          {c..c+4} (mod 8), gathered into a packed [256, 2560] featT_c.
          Chunks 0..3 are doubled on the host (matrix symmetry); chunk 4
          (the antipodal band) is computed by both paired cores, counted 1x.
          Label-equality + triangle masks precomputed as small uint8 inputs.
  device: G = gcorr + X^T X / B  (PE, over full batch from featrows)
          FGT = G @ featT_c; s_j row via ones-matmul; fgt2 = -2*FGT[:,loc];
          d2 tiles = (-2 X_loc G X^T + s_j) via 3-matmul PSUM accumulation;
          s_i via ACT bias; masked exp sums via ACT accum_out;
          min via DVE reduce_min.
  host:   combine per-core partial sums/mins -> final f32 scalar.
"""

import contextlib
import ctypes
import json
import os
import sys
import tempfile
import types

import ml_dtypes
import numpy as np

N_CORES = 8
B = 4096
D = 256
RPC = B // N_CORES  # rows per core = 512
MCH = RPC // 128  # m-chunks per core = 4
JW = 512  # column chunk width
MARGIN = 1.0
LAMBDA_CURV = 0.1
C_PARAM = 1.0
C0 = float(2**23)  # mask displacement

LAST_RUN_INFO = {}

# ----------------------------------------------------------------------------
# harness shims
# ----------------------------------------------------------------------------


def _split_waits_json(bir: dict) -> dict:
    n = [0]
    for f in bir.get("functions", []):
        for blk in f.get("blocks", []):
            out = []
            for inst in blk.get("instructions", []):
                si = inst.get("sync_info") or {}
                waits = si.get("on_wait") or []
                if len(waits) > 1:
                    for w in waits[1:]:
                        n[0] += 1
                        out.append(
                            {
                                "debug": inst.get("debug", 0),
                                "engine": inst["engine"],
                                "ins": [],
                                "name": f"I-waitsplit-{n[0]}",
                                "opcode": "NoOp",
                                "outs": [],
                                "sync_info": {"on_update": [], "on_wait": [w]},
                            }
                        )
                    si = dict(si)
                    si["on_wait"] = waits[:1]
                    inst = dict(inst)
                    inst["sync_info"] = si
                out.append(inst)
            blk["instructions"] = out
    return bir


def _patch_nc(nc):
    orig = nc.to_json_bytes

    def patched():
        return json.dumps(_split_waits_json(json.loads(orig()))).encode()

    nc.to_json_bytes = patched
    return nc


def _install_ntff_hook():
    if "antenv.axon_hooks" in sys.modules:
        return
    so_path = "/opt/axon/libaxon_pjrt.so"
    hook = None
    try:
        lib = ctypes.CDLL(so_path)
        if hasattr(lib, "axon_start_nrt_profile"):
            lib.axon_start_nrt_profile.argtypes = [
                ctypes.POINTER(ctypes.c_int64),
                ctypes.c_size_t,
            ]
            lib.axon_start_nrt_profile.restype = ctypes.c_int64
            lib.axon_stop_nrt_profile.argtypes = [ctypes.c_char_p]
            lib.axon_stop_nrt_profile.restype = ctypes.c_int64

            @contextlib.contextmanager
            def _hook(output_dir, device_ids):
                import jax

                jax.devices()
                if device_ids:
                    ids = (ctypes.c_int64 * len(device_ids))(*device_ids)
                    rc = lib.axon_start_nrt_profile(ids, len(device_ids))
                else:
                    rc = lib.axon_start_nrt_profile(None, 0)
                if rc != 0:
                    raise RuntimeError(f"axon_start_nrt_profile rc={rc}")
                try:
                    yield
                finally:
                    k = lib.axon_stop_nrt_profile(str(output_dir).encode())
                    print(f"profile: {k} file(s) -> {output_dir}", file=sys.stderr)

            hook = _hook
    except OSError:
        hook = None
    mod = types.ModuleType("antenv.axon_hooks")
    mod.get_axon_ntff_profile_hook = lambda: hook
    mod.set_axon_ntff_profile_hook = lambda h: None
    sys.modules["antenv.axon_hooks"] = mod


# ----------------------------------------------------------------------------
# device program
# ----------------------------------------------------------------------------


def _build_program(a_scale: float, nj: int, masked_js, mask_dt_np):
    """a_scale = -kappa > 0. nj: number of 512-col program chunks per core.
    masked_js: j chunks with label/triangle masks. mask_dt_np: mask np dtype."""
    from contextlib import ExitStack

    import concourse.bass as bass
    import concourse.tile as tile
    from concourse import mybir

    f32 = mybir.dt.float32
    f32r = mybir.dt.float32r
    mdt = mybir.dt.from_np(np.dtype(mask_dt_np))
    AF = mybir.ActivationFunctionType
    OP = mybir.AluOpType
    X = mybir.AxisListType.X

    a = float(a_scale)
    kappa = -a
    W = nj * JW  # program column width
    mw = len(masked_js) * JW  # mask width

    nc = bass.Bass("TRN2", target_bir_lowering=False, debug=False,
                   num_devices=N_CORES)

    featT = nc.dram_tensor("featT", [D, W], f32r, kind="ExternalInput").ap()
    featrows = nc.dram_tensor("featrows", [B, D], f32r, kind="ExternalInput").ap()
    amask = nc.dram_tensor("amask", [MCH, 128, mw], mdt, kind="ExternalInput").ap()
    bmask = nc.dram_tensor("bmask", [MCH, 128, mw], mdt, kind="ExternalInput").ap()
    gcorr = nc.dram_tensor("gcorr", [D, D], f32, kind="ExternalInput").ap()
    stats = nc.dram_tensor("stats", [128, 4], f32, kind="ExternalOutput").ap()
    sbounce = nc.dram_tensor("sbounce", [RPC], f32, kind="Internal").ap()

    with tile.TileContext(nc) as tc, ExitStack() as ctx:
        consts = ctx.enter_context(tc.tile_pool(name="consts", bufs=1))

        featT_sb = [consts.tile([128, W], f32r, name=f"featT{k}", tag=f"featT{k}")
                    for k in range(2)]
        am_sb = [consts.tile([128, mw], mdt, name=f"am{m}", tag=f"am{m}")
                 for m in range(MCH)]
        bm_sb = [consts.tile([128, mw], mdt, name=f"bm{m}", tag=f"bm{m}")
                 for m in range(MCH)]
        gcorr_sb = [consts.tile([128, D], f32, name=f"gc{k}", tag=f"gc{k}")
                    for k in range(2)]
        g_sb = [consts.tile([128, D], f32r, name=f"g{k}", tag=f"g{k}")
                for k in range(2)]
        fgt2 = [consts.tile([128, JW], f32r, name=f"fgt2{k}", tag=f"fgt2{k}")
                for k in range(2)]
        augR = consts.tile([1, W], f32r, name="augR", tag="augR")
        onescol = consts.tile([128, 1], f32r, name="onescol", tag="onescol")
        onesrow = consts.tile([1, 128], f32r, name="onesrow", tag="onesrow")
        scol4 = consts.tile([128, MCH], f32, name="scol4", tag="scol4")
        biasneg = consts.tile([128, MCH], f32, name="biasneg", tag="biasneg")
        biaspos = consts.tile([128, MCH], f32, name="biaspos", tag="biaspos")
        negacc = [consts.tile([128, nj], f32, name=f"negacc{m}", tag=f"negacc{m}")
                  for m in range(MCH)]
        posacc = [consts.tile([128, nj], f32, name=f"posacc{m}", tag=f"posacc{m}")
                  for m in range(MCH)]
        minacc = [consts.tile([128, nj], f32, name=f"minacc{m}", tag=f"minacc{m}")
                  for m in range(MCH)]
        stats_sb = consts.tile([128, 4], f32, name="stats_sb", tag="stats_sb")

        # PE warmup tile
        warm = consts.tile([128, JW], f32r, name="warm", tag="warm")
        nc.vector.memset(warm[:].bitcast(f32), 0.5)
        wdump = consts.tile([128, 1], f32, name="wdump", tag="wdump")

        # ---------------- DMAs: gcorr, frs, featT j01, masks, featT rest --
        for k in range(2):
            nc.gpsimd.dma_start(out=gcorr_sb[k][:],
                                in_=gcorr[k * 128:(k + 1) * 128, :])
        frs_ctx = tc.tile_pool(name="frs", bufs=4 if nj == 5 else 2)
        frs_pool = frs_ctx.__enter__()
        fr_r = featrows.rearrange("(g p) d -> g p d", p=128)  # [32,128,256]
        frs_tiles = []
        for blk in range(4):
            frs = frs_pool.tile([128, 8, D], f32r, name=f"frs{blk}",
                                tag=f"frs{blk}")
            nc.gpsimd.dma_start(
                out=frs[:],
                in_=fr_r[blk * 8:(blk + 1) * 8, :, :].rearrange("g p d -> p g d"),
            )
            frs_tiles.append(frs)
        for j in range(2):
            for k in range(2):
                nc.gpsimd.dma_start(
                    out=featT_sb[k][:, j * JW:(j + 1) * JW],
                    in_=featT[k * 128:(k + 1) * 128, j * JW:(j + 1) * JW],
                )
        for m in range(MCH):
            nc.gpsimd.dma_start(out=am_sb[m][:], in_=amask[m, :, :])
            nc.gpsimd.dma_start(out=bm_sb[m][:], in_=bmask[m, :, :])
        for j in range(2, nj):
            for k in range(2):
                nc.gpsimd.dma_start(
                    out=featT_sb[k][:, j * JW:(j + 1) * JW],
                    in_=featT[k * 128:(k + 1) * 128, j * JW:(j + 1) * JW],
                )

        nc.vector.memset(onescol[:].bitcast(f32), 1.0)
        nc.vector.memset(onesrow[:].bitcast(f32), 1.0)
        for m in range(MCH):
            nc.vector.memset(posacc[m][:], 0.0)

        # PE warmup while DMAs stream
        with tc.tile_pool(name="wps", bufs=1, space="PSUM") as wps_pool:
            wps = wps_pool.tile([128, JW], f32, name="wps", tag="wps")
            for w in range(20):
                nc.tensor.matmul(wps[:], lhsT=warm[:, 0:128], rhs=warm[:],
                                 start=(w == 0), stop=(w == 19),
                                 skip_group_check=True)
            nc.vector.tensor_reduce(out=wdump[:], in_=wps[:, 0:1],
                                    axis=X, op=OP.max)
        with tc.tile_pool(name="gps", bufs=2, space="PSUM") as gps_pool:
            g_ps = [gps_pool.tile([128, D], f32, name=f"gps{k}", tag=f"gps{k}")
                    for k in range(2)]
            for blk in range(4):
                frs = frs_tiles[blk]
                for g in range(8):
                    t = blk * 8 + g
                    xb = frs[:, g, :]
                    for kc in range(2):
                        nc.tensor.matmul(
                            g_ps[kc][:],
                            lhsT=xb[:, kc * 128:(kc + 1) * 128],
                            rhs=xb,
                            start=(t == 0),
                            stop=(t == 31),
                            skip_group_check=True,
                        )
            for kc in range(2):
                nc.vector.scalar_tensor_tensor(
                    out=g_sb[kc][:],
                    in0=g_ps[kc][:],
                    scalar=1.0 / B,
                    in1=gcorr_sb[kc][:],
                    op0=OP.mult,
                    op1=OP.add,
                )
        frs_ctx.__exit__(None, None, None)

        # ---------------- FGT, s-row, fgt2, s_col -------------------------
        prod_pool = ctx.enter_context(tc.tile_pool(name="prod", bufs=4))
        with tc.tile_pool(name="fps", bufs=3, space="PSUM") as fps_pool, \
             tc.tile_pool(name="sps", bufs=3, space="PSUM") as sps_pool:
            for j in range(nj):
                jsl = slice(j * JW, (j + 1) * JW)
                s_ps = sps_pool.tile([1, JW], f32, name="s_ps", tag="s_ps")
                for kc in range(2):
                    f_ps = fps_pool.tile([128, JW], f32, name="f_ps", tag="f_ps")
                    for ki in range(2):
                        nc.tensor.matmul(
                            f_ps[:],
                            lhsT=g_sb[ki][:, kc * 128:(kc + 1) * 128],
                            rhs=featT_sb[ki][:, jsl],
                            start=(ki == 0),
                            stop=(ki == 1),
                            skip_group_check=True,
                        )
                    if j == 0:
                        # local cols are program chunk 0
                        nc.scalar.mul(out=fgt2[kc][:],
                                      in_=f_ps[:], mul=-2.0)
                    prod = prod_pool.tile([128, JW], f32r, name="prod",
                                          tag="prod")
                    nc.vector.scalar_tensor_tensor(
                        out=prod[:], in0=f_ps[:], scalar=1.0,
                        in1=featT_sb[kc][:, jsl].bitcast(f32),
                        op0=OP.mult, op1=OP.mult)
                    nc.tensor.matmul(
                        s_ps[:],
                        lhsT=onescol[:],
                        rhs=prod[:],
                        start=(kc == 0),
                        stop=(kc == 1),
                        skip_group_check=True,
                    )
                nc.scalar.copy(out=augR[0:1, jsl], in_=s_ps[:])
                if j == 0:
                    nc.gpsimd.dma_start(out=sbounce[:],
                                        in_=augR[0:1, 0:RPC].bitcast(f32))
                    nc.gpsimd.dma_start(
                        out=scol4[:],
                        in_=bass.AP(sbounce.tensor, 0, [[1, 128], [128, MCH]]),
                    )
                    nc.vector.tensor_scalar_mul(out=biasneg[:], in0=scol4[:],
                                                scalar1=a)
                    nc.vector.tensor_scalar_mul(out=biaspos[:], in0=scol4[:],
                                                scalar1=kappa)

        # ---------------- main pairwise tiles -----------------------------
        scr_pool = ctx.enter_context(tc.tile_pool(name="scr", bufs=3))
        dump_pool = ctx.enter_context(tc.tile_pool(name="dump", bufs=4))
        with tc.tile_pool(name="mps", bufs=8, space="PSUM") as mps_pool:
            for j in range(nj):
                jsl = slice(j * JW, (j + 1) * JW)
                for m in range(MCH):
                    ps = mps_pool.tile([128, JW], f32, name="ps", tag="ps")
                    for ki in range(2):
                        nc.tensor.matmul(
                            ps[:],
                            lhsT=fgt2[ki][:, m * 128:(m + 1) * 128],
                            rhs=featT_sb[ki][:, jsl],
                            start=(ki == 0),
                            stop=False,
                            skip_group_check=True,
                        )
                    nc.tensor.matmul(
                        ps[:],
                        lhsT=onesrow[:],
                        rhs=augR[0:1, jsl],
                        start=False,
                        stop=True,
                        skip_group_check=True,
                    )
                    bneg = biasneg[:, m:m + 1]
                    bpos = biaspos[:, m:m + 1]
                    if j in masked_js:
                        jj = masked_js.index(j)
                        at = am_sb[m][:, jj * JW:(jj + 1) * JW]
                        bt = bm_sb[m][:, jj * JW:(jj + 1) * JW]
                        q = scr_pool.tile([128, JW], f32, name="q", tag="q")
                        nc.vector.scalar_tensor_tensor(
                            out=q[:], in0=at, scalar=C0, in1=ps[:],
                            op0=OP.mult, op1=OP.subtract)
                        tmin = scr_pool.tile([128, JW], f32, name="tmin",
                                             tag="tmin")
                        nc.vector.scalar_tensor_tensor(
                            out=tmin[:], in0=at, scalar=C0, in1=ps[:],
                            op0=OP.mult, op1=OP.add)
                        tpos = scr_pool.tile([128, JW], f32, name="tpos",
                                             tag="tpos")
                        nc.vector.scalar_tensor_tensor(
                            out=tpos[:], in0=bt, scalar=C0, in1=ps[:],
                            op0=OP.mult, op1=OP.add)
                        dump = dump_pool.tile([128, JW], f32, name="dump",
                                              tag="dump")
                        nc.scalar.activation(
                            out=dump[:], in_=q[:], func=AF.Exp, scale=kappa,
                            bias=bneg, accum_out=negacc[m][:, j:j + 1])
                        dump2 = dump_pool.tile([128, JW], f32, name="dump",
                                               tag="dump")
                        nc.scalar.activation(
                            out=dump2[:], in_=tpos[:], func=AF.Exp, scale=kappa,
                            bias=bpos, accum_out=posacc[m][:, j:j + 1])
                        nc.vector.tensor_reduce(
                            out=minacc[m][:, j:j + 1], in_=tmin[:], axis=X,
                            op=OP.min)
                    else:
                        dump = dump_pool.tile([128, JW], f32, name="dump",
                                              tag="dump")
                        nc.scalar.activation(
                            out=dump[:], in_=ps[:], func=AF.Exp, scale=a,
                            bias=bneg, accum_out=negacc[m][:, j:j + 1])
                        nc.vector.tensor_reduce(
                            out=minacc[m][:, j:j + 1], in_=ps[:], axis=X,
                            op=OP.min)

        # ---------------- finalize ----------------------------------------
        # stats: col0 = neg sum j<nj-1 (doubled on host), col1 = neg j=nj-1,
        #        col2 = pos sum, col3 = min d2
        fin = ctx.enter_context(tc.tile_pool(name="fin", bufs=1))
        negrow = fin.tile([128, MCH], f32, name="negrow", tag="negrow")
        negrow4 = fin.tile([128, MCH], f32, name="negrow4", tag="negrow4")
        posrow = fin.tile([128, MCH], f32, name="posrow", tag="posrow")
        minrow = fin.tile([128, MCH], f32, name="minrow", tag="minrow")
        for m in range(MCH):
            nc.vector.tensor_reduce(out=negrow[:, m:m + 1],
                                    in_=negacc[m][:, 0:nj - 1], axis=X,
                                    op=OP.add)
            nc.vector.tensor_copy(out=negrow4[:, m:m + 1],
                                  in_=negacc[m][:, nj - 1:nj])
            nc.vector.tensor_reduce(out=posrow[:, m:m + 1], in_=posacc[m][:],
                                    axis=X, op=OP.add)
            rm = fin.tile([128, 1], f32, name=f"rm{m}", tag=f"rm{m}")
            nc.vector.tensor_reduce(out=rm[:], in_=minacc[m][:], axis=X,
                                    op=OP.min)
            nc.vector.tensor_add(out=minrow[:, m:m + 1], in0=rm[:],
                                 in1=scol4[:, m:m + 1])
        nc.vector.tensor_reduce(out=stats_sb[:, 0:1], in_=negrow[:], axis=X,
                                op=OP.add)
        nc.vector.tensor_reduce(out=stats_sb[:, 1:2], in_=negrow4[:], axis=X,
                                op=OP.add)
        nc.vector.tensor_reduce(out=stats_sb[:, 2:3], in_=posrow[:], axis=X,
                                op=OP.add)
        nc.vector.tensor_reduce(out=stats_sb[:, 3:4], in_=minrow[:], axis=X,
                                op=OP.min)
        nc.sync.dma_start(out=stats, in_=stats_sb[:])

    return _patch_nc(nc)


# ----------------------------------------------------------------------------
# host wrapper
# ----------------------------------------------------------------------------


def kernel(features, w1, b1, w2, b2, kappa_param, labels):
    features = np.asarray(features, dtype=np.float32)
    w1 = np.asarray(w1, dtype=np.float32)
    b1 = np.asarray(b1, dtype=np.float32)
    w2 = np.asarray(w2, dtype=np.float32)
    b2 = np.asarray(b2, dtype=np.float32)
    kappa_param = np.float32(np.asarray(kappa_param))
    labels_i = np.asarray(labels).astype(np.int64)

    assert features.shape == (B, D)

    # ---- tiny MLP -> kappa ----
    mu = features.mean(axis=0, dtype=np.float32).astype(np.float32)
    h = np.tanh(mu @ w1 + b1).astype(np.float32)
    z = np.float32((h @ w2 + b2)[0])
    softplus = np.float32(np.logaddexp(np.float32(0.0), z))
    kappa = np.float32(-softplus)
    a = float(-kappa)

    _, counts = np.unique(labels_i, return_counts=True)
    n_pos = int((counts * (counts - 1)).sum())
    n_neg = int(B * (B - 1) - n_pos)

    order = np.argsort(labels_i, kind="stable")
    Xs = np.ascontiguousarray(features[order])
    ls = labels_i[order]
    featT_s = np.ascontiguousarray(Xs.T)  # [D, B]

    gcorr = (np.eye(D, dtype=np.float32) - np.outer(mu, mu)).astype(np.float32)
    mask_np = np.uint8

    # triangle exclusion (chunk 0): excl iff q <= 128m + p
    qs = np.arange(JW)
    tri = np.zeros((MCH, 128, JW), dtype=np.int32)
    for m in range(MCH):
        for p in range(128):
            tri[m, p] = (qs <= 128 * m + p)

    # symmetric mode: core c -> global chunks {c..c+4}; masked {0,1}.
    # Same-label cells of core c's rows must lie in global cols
    # [512c - 512, 512c + 1024): below-diagonal part is triangle/mirror
    # covered, the rest must fall in the two masked chunks.
    nj = 5
    masked_js = [0, 1]
    sym_ok = True
    for c in range(N_CORES):
        rows = ls[c * RPC:(c + 1) * RPC]
        inside = np.zeros(B, dtype=bool)
        for g in range(c * RPC - RPC, c * RPC + 1024):
            inside[g % B] = True
        if (rows[:, None] == ls[None, inside == False]).any():  # noqa: E712
            sym_ok = False
            break

    if not sym_ok:
        nj = N_CORES
        masked_js = list(range(nj))
    mw = len(masked_js) * JW
    dbl = np.float32(2.0) if sym_ok else np.float32(1.0)

    in_maps = []
    for c in range(N_CORES):
        gchunks = [(c + t) % N_CORES for t in range(nj)]
        cols = np.concatenate([np.arange(g * JW, (g + 1) * JW)
                               for g in gchunks])
        featT_c = np.ascontiguousarray(featT_s[:, cols])
        rows = ls[c * RPC:(c + 1) * RPC]
        lcols = ls[cols[:mw]]
        am = np.zeros((MCH, 128, mw), dtype=np.int32)
        bm = np.zeros((MCH, 128, mw), dtype=np.int32)
        for m in range(MCH):
            r = rows[m * 128:(m + 1) * 128]
            eq = (r[:, None] == lcols[None, :]).astype(np.int32)
            am[m] = eq
            bm[m] = 1 - eq
            if sym_ok:
                am[m, :, :JW] += tri[m]
                bm[m, :, :JW] += tri[m]
            else:
                # exclude the diagonal from pos explicitly
                for p in range(128):
                    g = c * RPC + m * 128 + p
                    pos = int(np.where(cols == g)[0][0])
                    bm[m, p, pos] += 1
        in_maps.append(
            {
                "featT": featT_c,
                "featrows": Xs,
                "amask": am.astype(mask_np),
                "bmask": bm.astype(mask_np),
                "gcorr": gcorr,
            }
        )

    trace = bool(os.environ.get("BASS_TRACE"))
    if trace:
        _install_ntff_hook()
    nc = _build_program(a, nj, masked_js, mask_np)

    from concourse.bass_utils import run_bass_kernel_spmd

    kwargs = {}
    if trace:
        kwargs["tmpdir"] = tempfile.mkdtemp(prefix="curvloss_trace_")
    res = run_bass_kernel_spmd(nc, in_maps, core_ids=list(range(N_CORES)),
                               trace=trace, **kwargs)
    LAST_RUN_INFO.clear()
    LAST_RUN_INFO.update(
        exec_time_ns=res.exec_time_ns,
        min_d2=None,
        mean_exec_time_ns=res.mean_exec_time_ns,
        trace=res.instructions_and_trace[1] if res.instructions_and_trace else None,
        tmpdir=kwargs.get("tmpdir"),
    )

    allstats = np.stack([res.results[c]["stats"] for c in range(N_CORES)])
    f32 = np.float32
    neg_a = f32(allstats[:, :, 0].sum(dtype=np.float32))
    neg_b = f32(allstats[:, :, 1].sum(dtype=np.float32))
    negsum = f32(dbl * neg_a + neg_b)
    possum = f32(dbl * f32(allstats[:, :, 2].sum(dtype=np.float32)))
    min_d2 = f32(allstats[:, :, 3].min())

    LAST_RUN_INFO["min_d2"] = float(min_d2)
    positive_loss = f32(possum / f32(max(n_pos, 1)))
    negative_loss = f32(negsum / f32(max(n_neg, 1)))
    contrastive = f32(positive_loss - negative_loss + f32(MARGIN))

    d2r = f32(max(min_d2, f32(0.0)))
    d2r = f32(max(d2r, f32(1e-12)))
    min_inter = f32(np.sqrt(d2r))
    delta = f32(max(min_inter, f32(0.1)))
    constraint = f32(-C_PARAM / (delta * delta))
    reg = f32(LAMBDA_CURV * max(f32(0.0), f32(kappa_param - constraint)))
    out = f32(contrastive + reg)
    return np.asarray(out, dtype=np.float32)


# revision 16
# speedup vs baseline: 1.1200x; 1.1200x over previous
"""Trainium2 Bass kernel for nn_CurvatureAwareLoss.

Strategy (8 NeuronCores, symmetric data-parallel over batch rows):
  host:   sort rows by label. Core c owns rows [512c, 512c+512) and the
          upper-triangle-ish band of columns: global 512-col chunks
          {c..c+4} (mod 8), gathered into a packed [256, 2560] featT_c.
          Chunks 0..3 are doubled on the host (matrix symmetry); chunk 4
          (the antipodal band) is computed by both paired cores, counted 1x.
          Label-equality + triangle masks precomputed as small uint8 inputs.
  device: G = gcorr + X^T X / B  (PE, over full batch from featrows)
          FGT = G @ featT_c; s_j row via ones-matmul; fgt2 = -2*FGT[:,loc];
          d2 tiles = (-2 X_loc G X^T + s_j) via 3-matmul PSUM accumulation;
          s_i via ACT bias; masked exp sums via ACT accum_out;
          min via DVE reduce_min.
  host:   combine per-core partial sums/mins -> final f32 scalar.
"""

import contextlib
import ctypes
import json
import os
import sys
import tempfile
import types

import ml_dtypes
import numpy as np

N_CORES = 8
B = 4096
D = 256
RPC = B // N_CORES  # rows per core = 512
MCH = RPC // 128  # m-chunks per core = 4
JW = 512  # column chunk width
MARGIN = 1.0
LAMBDA_CURV = 0.1
C_PARAM = 1.0
C0 = float(2**23)  # mask displacement

LAST_RUN_INFO = {}

# ----------------------------------------------------------------------------
# harness shims
# ----------------------------------------------------------------------------


def _split_waits_json(bir: dict) -> dict:
    n = [0]
    for f in bir.get("functions", []):
        for blk in f.get("blocks", []):
            out = []
            for inst in blk.get("instructions", []):
                si = inst.get("sync_info") or {}
                waits = si.get("on_wait") or []
                if len(waits) > 1:
                    for w in waits[1:]:
                        n[0] += 1
                        out.append(
                            {
                                "debug": inst.get("debug", 0),
                                "engine": inst["engine"],
                                "ins": [],
                                "name": f"I-waitsplit-{n[0]}",
                                "opcode": "NoOp",
                                "outs": [],
                                "sync_info": {"on_update": [], "on_wait": [w]},
                            }
                        )
                    si = dict(si)
                    si["on_wait"] = waits[:1]
                    inst = dict(inst)
                    inst["sync_info"] = si
                out.append(inst)
            blk["instructions"] = out
    return bir


def _patch_nc(nc):
    orig = nc.to_json_bytes

    def patched():
        return json.dumps(_split_waits_json(json.loads(orig()))).encode()

    nc.to_json_bytes = patched
    return nc


def _install_ntff_hook():
    if "antenv.axon_hooks" in sys.modules:
        return
    so_path = "/opt/axon/libaxon_pjrt.so"
    hook = None
    try:
        lib = ctypes.CDLL(so_path)
        if hasattr(lib, "axon_start_nrt_profile"):
            lib.axon_start_nrt_profile.argtypes = [
                ctypes.POINTER(ctypes.c_int64),
                ctypes.c_size_t,
            ]
            lib.axon_start_nrt_profile.restype = ctypes.c_int64
            lib.axon_stop_nrt_profile.argtypes = [ctypes.c_char_p]
            lib.axon_stop_nrt_profile.restype = ctypes.c_int64

            @contextlib.contextmanager
            def _hook(output_dir, device_ids):
                import jax

                jax.devices()
                if device_ids:
                    ids = (ctypes.c_int64 * len(device_ids))(*device_ids)
                    rc = lib.axon_start_nrt_profile(ids, len(device_ids))
                else:
                    rc = lib.axon_start_nrt_profile(None, 0)
                if rc != 0:
                    raise RuntimeError(f"axon_start_nrt_profile rc={rc}")
                try:
                    yield
                finally:
                    k = lib.axon_stop_nrt_profile(str(output_dir).encode())
                    print(f"profile: {k} file(s) -> {output_dir}", file=sys.stderr)

            hook = _hook
    except OSError:
        hook = None
    mod = types.ModuleType("antenv.axon_hooks")
    mod.get_axon_ntff_profile_hook = lambda: hook
    mod.set_axon_ntff_profile_hook = lambda h: None
    sys.modules["antenv.axon_hooks"] = mod


# ----------------------------------------------------------------------------
# device program
# ----------------------------------------------------------------------------


def _build_program(a_scale: float, nj: int, masked_js, mask_dt_np):
    """a_scale = -kappa > 0. nj: number of 512-col program chunks per core.
    masked_js: j chunks with label/triangle masks. mask_dt_np: mask np dtype."""
    from contextlib import ExitStack

    import concourse.bass as bass
    import concourse.tile as tile
    from concourse import mybir

    f32 = mybir.dt.float32
    f32r = mybir.dt.float32r
    mdt = mybir.dt.from_np(np.dtype(mask_dt_np))
    AF = mybir.ActivationFunctionType
    OP = mybir.AluOpType
    X = mybir.AxisListType.X

    a = float(a_scale)
    kappa = -a
    W = nj * JW  # program column width
    mw = len(masked_js) * JW  # mask width

    nc = bass.Bass("TRN2", target_bir_lowering=False, debug=False,
                   num_devices=N_CORES)

    featT = nc.dram_tensor("featT", [D, W], f32r, kind="ExternalInput").ap()
    featrows = nc.dram_tensor("featrows", [B, D], f32r, kind="ExternalInput").ap()
    amask = nc.dram_tensor("amask", [MCH, 128, mw], mdt, kind="ExternalInput").ap()
    bmask = nc.dram_tensor("bmask", [MCH, 128, mw], mdt, kind="ExternalInput").ap()
    gcorr = nc.dram_tensor("gcorr", [D, D], f32, kind="ExternalInput").ap()
    stats = nc.dram_tensor("stats", [128, 4], f32, kind="ExternalOutput").ap()
    sbounce = nc.dram_tensor("sbounce", [RPC], f32, kind="Internal").ap()

    with tile.TileContext(nc) as tc, ExitStack() as ctx:
        consts = ctx.enter_context(tc.tile_pool(name="consts", bufs=1))

        featT_sb = [consts.tile([128, W], f32r, name=f"featT{k}", tag=f"featT{k}")
                    for k in range(2)]
        am_sb = [consts.tile([128, mw], mdt, name=f"am{m}", tag=f"am{m}")
                 for m in range(MCH)]
        bm_sb = [consts.tile([128, mw], mdt, name=f"bm{m}", tag=f"bm{m}")
                 for m in range(MCH)]
        gcorr_sb = [consts.tile([128, D], f32, name=f"gc{k}", tag=f"gc{k}")
                    for k in range(2)]
        g_sb = [consts.tile([128, D], f32r, name=f"g{k}", tag=f"g{k}")
                for k in range(2)]
        fgt2 = [consts.tile([128, JW], f32r, name=f"fgt2{k}", tag=f"fgt2{k}")
                for k in range(2)]
        augR = consts.tile([1, W], f32r, name="augR", tag="augR")
        onescol = consts.tile([128, 1], f32r, name="onescol", tag="onescol")
        onesrow = consts.tile([1, 128], f32r, name="onesrow", tag="onesrow")
        scol4 = consts.tile([128, MCH], f32, name="scol4", tag="scol4")
        biasneg = consts.tile([128, MCH], f32, name="biasneg", tag="biasneg")
        biaspos = consts.tile([128, MCH], f32, name="biaspos", tag="biaspos")
        negacc = [consts.tile([128, nj], f32, name=f"negacc{m}", tag=f"negacc{m}")
                  for m in range(MCH)]
        posacc = [consts.tile([128, nj], f32, name=f"posacc{m}", tag=f"posacc{m}")
                  for m in range(MCH)]
        minacc = [consts.tile([128, nj], f32, name=f"minacc{m}", tag=f"minacc{m}")
                  for m in range(MCH)]
        stats_sb = consts.tile([128, 4], f32, name="stats_sb", tag="stats_sb")

        # PE warmup tile
        warm = consts.tile([128, JW], f32r, name="warm", tag="warm")
        nc.vector.memset(warm[:].bitcast(f32), 0.5)
        wdump = consts.tile([128, 1], f32, name="wdump", tag="wdump")

        # ---------------- DMAs: gcorr, frs, featT j01, masks, featT rest --
        for k in range(2):
            nc.gpsimd.dma_start(out=gcorr_sb[k][:],
                                in_=gcorr[k * 128:(k + 1) * 128, :])
        frs_ctx = tc.tile_pool(name="frs", bufs=4 if nj == 5 else 2)
        frs_pool = frs_ctx.__enter__()
        fr_r = featrows.rearrange("(g p) d -> g p d", p=128)  # [32,128,256]
        frs_tiles = []
        for blk in range(4):
            frs = frs_pool.tile([128, 8, D], f32r, name=f"frs{blk}",
                                tag=f"frs{blk}")
            nc.gpsimd.dma_start(
                out=frs[:],
                in_=fr_r[blk * 8:(blk + 1) * 8, :, :].rearrange("g p d -> p g d"),
            )
            frs_tiles.append(frs)
        for j in range(2):
            for k in range(2):
                nc.gpsimd.dma_start(
                    out=featT_sb[k][:, j * JW:(j + 1) * JW],
                    in_=featT[k * 128:(k + 1) * 128, j * JW:(j + 1) * JW],
                )
        for m in range(MCH):
            nc.gpsimd.dma_start(out=am_sb[m][:], in_=amask[m, :, :])
            nc.gpsimd.dma_start(out=bm_sb[m][:], in_=bmask[m, :, :])
        for j in range(2, nj):
            for k in range(2):
                nc.gpsimd.dma_start(
                    out=featT_sb[k][:, j * JW:(j + 1) * JW],
                    in_=featT[k * 128:(k + 1) * 128, j * JW:(j + 1) * JW],
                )

        nc.vector.memset(onescol[:].bitcast(f32), 1.0)
        nc.vector.memset(onesrow[:].bitcast(f32), 1.0)
        for m in range(MCH):
            nc.vector.memset(posacc[m][:], 0.0)

        # PE warmup while DMAs stream
        with tc.tile_pool(name="wps", bufs=1, space="PSUM") as wps_pool:
            wps = wps_pool.tile([128, JW], f32, name="wps", tag="wps")
            for w in range(20):
                nc.tensor.matmul(wps[:], lhsT=warm[:, 0:128], rhs=warm[:],
                                 start=(w == 0), stop=(w == 19),
                                 skip_group_check=True)
            nc.vector.tensor_reduce(out=wdump[:], in_=wps[:, 0:1],
                                    axis=X, op=OP.max)
        with tc.tile_pool(name="gps", bufs=2, space="PSUM") as gps_pool:
            g_ps = [gps_pool.tile([128, D], f32, name=f"gps{k}", tag=f"gps{k}")
                    for k in range(2)]
            for blk in range(4):
                frs = frs_tiles[blk]
                for g in range(8):
                    t = blk * 8 + g
                    xb = frs[:, g, :]
                    for kc in range(2):
                        nc.tensor.matmul(
                            g_ps[kc][:],
                            lhsT=xb[:, kc * 128:(kc + 1) * 128],
                            rhs=xb,
                            start=(t == 0),
                            stop=(t == 31),
                            skip_group_check=True,
                        )
            for kc in range(2):
                nc.vector.scalar_tensor_tensor(
                    out=g_sb[kc][:],
                    in0=g_ps[kc][:],
                    scalar=1.0 / B,
                    in1=gcorr_sb[kc][:],
                    op0=OP.mult,
                    op1=OP.add,
                )
        frs_ctx.__exit__(None, None, None)

        # ---------------- FGT, s-row, fgt2, s_col -------------------------
        prod_pool = ctx.enter_context(tc.tile_pool(name="prod", bufs=4))
        with tc.tile_pool(name="fps", bufs=3, space="PSUM") as fps_pool, \
             tc.tile_pool(name="sps", bufs=3, space="PSUM") as sps_pool:
            for j in range(nj):
                jsl = slice(j * JW, (j + 1) * JW)
                s_ps = sps_pool.tile([1, JW], f32, name="s_ps", tag="s_ps")
                for kc in range(2):
                    f_ps = fps_pool.tile([128, JW], f32, name="f_ps", tag="f_ps")
                    for ki in range(2):
                        nc.tensor.matmul(
                            f_ps[:],
                            lhsT=g_sb[ki][:, kc * 128:(kc + 1) * 128],
                            rhs=featT_sb[ki][:, jsl],
                            start=(ki == 0),
                            stop=(ki == 1),
                            skip_group_check=True,
                        )
                    if j == 0:
                        # local cols are program chunk 0
                        nc.scalar.mul(out=fgt2[kc][:],
                                      in_=f_ps[:], mul=-2.0)
                    prod = prod_pool.tile([128, JW], f32r, name="prod",
                                          tag="prod")
                    nc.vector.scalar_tensor_tensor(
                        out=prod[:], in0=f_ps[:], scalar=1.0,
                        in1=featT_sb[kc][:, jsl].bitcast(f32),
                        op0=OP.mult, op1=OP.mult)
                    nc.tensor.matmul(
                        s_ps[:],
                        lhsT=onescol[:],
                        rhs=prod[:],
                        start=(kc == 0),
                        stop=(kc == 1),
                        skip_group_check=True,
                    )
                nc.scalar.copy(out=augR[0:1, jsl], in_=s_ps[:])
                if j == 0:
                    nc.gpsimd.dma_start(out=sbounce[:],
                                        in_=augR[0:1, 0:RPC].bitcast(f32))
                    nc.gpsimd.dma_start(
                        out=scol4[:],
                        in_=bass.AP(sbounce.tensor, 0, [[1, 128], [128, MCH]]),
                    )
                    nc.vector.tensor_scalar_mul(out=biasneg[:], in0=scol4[:],
                                                scalar1=a)
                    nc.vector.tensor_scalar_mul(out=biaspos[:], in0=scol4[:],
                                                scalar1=kappa)

        # ---------------- main pairwise tiles -----------------------------
        scr_pool = ctx.enter_context(tc.tile_pool(name="scr", bufs=3))
        dump_pool = ctx.enter_context(tc.tile_pool(name="dump", bufs=4))
        masked_pairs = [(m, j) for j in masked_js for m in range(MCH)]
        clean_pairs = [(m, j) for j in range(nj) if j not in masked_js
                       for m in range(MCH)]
        tile_order = []
        ci = 0
        for mp in masked_pairs:
            tile_order.append(mp)
            if ci < len(clean_pairs):
                tile_order.append(clean_pairs[ci])
                ci += 1
        tile_order.extend(clean_pairs[ci:])
        with tc.tile_pool(name="mps", bufs=8, space="PSUM") as mps_pool:
            for m, j in tile_order:
                jsl = slice(j * JW, (j + 1) * JW)
                if True:
                    ps = mps_pool.tile([128, JW], f32, name="ps", tag="ps")
                    for ki in range(2):
                        nc.tensor.matmul(
                            ps[:],
                            lhsT=fgt2[ki][:, m * 128:(m + 1) * 128],
                            rhs=featT_sb[ki][:, jsl],
                            start=(ki == 0),
                            stop=False,
                            skip_group_check=True,
                        )
                    nc.tensor.matmul(
                        ps[:],
                        lhsT=onesrow[:],
                        rhs=augR[0:1, jsl],
                        start=False,
                        stop=True,
                        skip_group_check=True,
                    )
                    bneg = biasneg[:, m:m + 1]
                    bpos = biaspos[:, m:m + 1]
                    if j in masked_js:
                        jj = masked_js.index(j)
                        at = am_sb[m][:, jj * JW:(jj + 1) * JW]
                        bt = bm_sb[m][:, jj * JW:(jj + 1) * JW]
                        q = scr_pool.tile([128, JW], f32, name="q", tag="q")
                        nc.vector.scalar_tensor_tensor(
                            out=q[:], in0=at, scalar=C0, in1=ps[:],
                            op0=OP.mult, op1=OP.subtract)
                        tmin = scr_pool.tile([128, JW], f32, name="tmin",
                                             tag="tmin")
                        nc.vector.scalar_tensor_tensor(
                            out=tmin[:], in0=at, scalar=C0, in1=ps[:],
                            op0=OP.mult, op1=OP.add)
                        tpos = scr_pool.tile([128, JW], f32, name="tpos",
                                             tag="tpos")
                        nc.vector.scalar_tensor_tensor(
                            out=tpos[:], in0=bt, scalar=C0, in1=ps[:],
                            op0=OP.mult, op1=OP.add)
                        dump = dump_pool.tile([128, JW], f32, name="dump",
                                              tag="dump")
                        nc.scalar.activation(
                            out=dump[:], in_=q[:], func=AF.Exp, scale=kappa,
                            bias=bneg, accum_out=negacc[m][:, j:j + 1])
                        dump2 = dump_pool.tile([128, JW], f32, name="dump",
                                               tag="dump")
                        nc.scalar.activation(
                            out=dump2[:], in_=tpos[:], func=AF.Exp, scale=kappa,
                            bias=bpos, accum_out=posacc[m][:, j:j + 1])
                        nc.vector.tensor_reduce(
                            out=minacc[m][:, j:j + 1], in_=tmin[:], axis=X,
                            op=OP.min)
                    else:
                        dump = dump_pool.tile([128, JW], f32, name="dump",
                                              tag="dump")
                        nc.scalar.activation(
                            out=dump[:], in_=ps[:], func=AF.Exp, scale=a,
                            bias=bneg, accum_out=negacc[m][:, j:j + 1])
                        nc.vector.tensor_reduce(
                            out=minacc[m][:, j:j + 1], in_=ps[:], axis=X,
                            op=OP.min)

        # ---------------- finalize ----------------------------------------
        # stats: col0 = neg sum j<nj-1 (doubled on host), col1 = neg j=nj-1,
        #        col2 = pos sum, col3 = min d2
        fin = ctx.enter_context(tc.tile_pool(name="fin", bufs=1))
        negrow = fin.tile([128, MCH], f32, name="negrow", tag="negrow")
        negrow4 = fin.tile([128, MCH], f32, name="negrow4", tag="negrow4")
        posrow = fin.tile([128, MCH], f32, name="posrow", tag="posrow")
        minrow = fin.tile([128, MCH], f32, name="minrow", tag="minrow")
        for m in range(MCH):
            nc.vector.tensor_reduce(out=negrow[:, m:m + 1],
                                    in_=negacc[m][:, 0:nj - 1], axis=X,
                                    op=OP.add)
            nc.vector.tensor_copy(out=negrow4[:, m:m + 1],
                                  in_=negacc[m][:, nj - 1:nj])
            nc.vector.tensor_reduce(out=posrow[:, m:m + 1], in_=posacc[m][:],
                                    axis=X, op=OP.add)
            rm = fin.tile([128, 1], f32, name=f"rm{m}", tag=f"rm{m}")
            nc.vector.tensor_reduce(out=rm[:], in_=minacc[m][:], axis=X,
                                    op=OP.min)
            nc.vector.tensor_add(out=minrow[:, m:m + 1], in0=rm[:],
                                 in1=scol4[:, m:m + 1])
        nc.vector.tensor_reduce(out=stats_sb[:, 0:1], in_=negrow[:], axis=X,
                                op=OP.add)
        nc.vector.tensor_reduce(out=stats_sb[:, 1:2], in_=negrow4[:], axis=X,
                                op=OP.add)
        nc.vector.tensor_reduce(out=stats_sb[:, 2:3], in_=posrow[:], axis=X,
                                op=OP.add)
        nc.vector.tensor_reduce(out=stats_sb[:, 3:4], in_=minrow[:], axis=X,
                                op=OP.min)
        nc.sync.dma_start(out=stats, in_=stats_sb[:])

    return _patch_nc(nc)


# ----------------------------------------------------------------------------
# host wrapper
# ----------------------------------------------------------------------------


def kernel(features, w1, b1, w2, b2, kappa_param, labels):
    features = np.asarray(features, dtype=np.float32)
    w1 = np.asarray(w1, dtype=np.float32)
    b1 = np.asarray(b1, dtype=np.float32)
    w2 = np.asarray(w2, dtype=np.float32)
    b2 = np.asarray(b2, dtype=np.float32)
    kappa_param = np.float32(np.asarray(kappa_param))
    labels_i = np.asarray(labels).astype(np.int64)

    assert features.shape == (B, D)

    # ---- tiny MLP -> kappa ----
    mu = features.mean(axis=0, dtype=np.float32).astype(np.float32)
    h = np.tanh(mu @ w1 + b1).astype(np.float32)
    z = np.float32((h @ w2 + b2)[0])
    softplus = np.float32(np.logaddexp(np.float32(0.0), z))
    kappa = np.float32(-softplus)
    a = float(-kappa)

    _, counts = np.unique(labels_i, return_counts=True)
    n_pos = int((counts * (counts - 1)).sum())
    n_neg = int(B * (B - 1) - n_pos)

    order = np.argsort(labels_i, kind="stable")
    Xs = np.ascontiguousarray(features[order])
    ls = labels_i[order]
    featT_s = np.ascontiguousarray(Xs.T)  # [D, B]

    gcorr = (np.eye(D, dtype=np.float32) - np.outer(mu, mu)).astype(np.float32)
    mask_np = np.uint8

    # triangle exclusion (chunk 0): excl iff q <= 128m + p
    qs = np.arange(JW)
    tri = np.zeros((MCH, 128, JW), dtype=np.int32)
    for m in range(MCH):
        for p in range(128):
            tri[m, p] = (qs <= 128 * m + p)

    # symmetric mode: core c -> global chunks {c..c+4}; masked {0,1}.
    # Same-label cells of core c's rows must lie in global cols
    # [512c - 512, 512c + 1024): below-diagonal part is triangle/mirror
    # covered, the rest must fall in the two masked chunks.
    nj = 5
    masked_js = [0, 1]
    sym_ok = True
    for c in range(N_CORES):
        rows = ls[c * RPC:(c + 1) * RPC]
        inside = np.zeros(B, dtype=bool)
        for g in range(c * RPC - RPC, c * RPC + 1024):
            inside[g % B] = True
        if (rows[:, None] == ls[None, inside == False]).any():  # noqa: E712
            sym_ok = False
            break

    if not sym_ok:
        nj = N_CORES
        masked_js = list(range(nj))
    mw = len(masked_js) * JW
    dbl = np.float32(2.0) if sym_ok else np.float32(1.0)

    in_maps = []
    for c in range(N_CORES):
        gchunks = [(c + t) % N_CORES for t in range(nj)]
        cols = np.concatenate([np.arange(g * JW, (g + 1) * JW)
                               for g in gchunks])
        featT_c = np.ascontiguousarray(featT_s[:, cols])
        rows = ls[c * RPC:(c + 1) * RPC]
        lcols = ls[cols[:mw]]
        am = np.zeros((MCH, 128, mw), dtype=np.int32)
        bm = np.zeros((MCH, 128, mw), dtype=np.int32)
        for m in range(MCH):
            r = rows[m * 128:(m + 1) * 128]
            eq = (r[:, None] == lcols[None, :]).astype(np.int32)
            am[m] = eq
            bm[m] = 1 - eq
            if sym_ok:
                am[m, :, :JW] += tri[m]
                bm[m, :, :JW] += tri[m]
            else:
                # exclude the diagonal from pos explicitly
                for p in range(128):
                    g = c * RPC + m * 128 + p
                    pos = int(np.where(cols == g)[0][0])
                    bm[m, p, pos] += 1
        in_maps.append(
            {
                "featT": featT_c,
                "featrows": Xs,
                "amask": am.astype(mask_np),
                "bmask": bm.astype(mask_np),
                "gcorr": gcorr,
            }
        )

    trace = bool(os.environ.get("BASS_TRACE"))
    if trace:
        _install_ntff_hook()
    nc = _build_program(a, nj, masked_js, mask_np)

    from concourse.bass_utils import run_bass_kernel_spmd

    kwargs = {}
    if trace:
        kwargs["tmpdir"] = tempfile.mkdtemp(prefix="curvloss_trace_")
    res = run_bass_kernel_spmd(nc, in_maps, core_ids=list(range(N_CORES)),
                               trace=trace, **kwargs)
    LAST_RUN_INFO.clear()
    LAST_RUN_INFO.update(
        exec_time_ns=res.exec_time_ns,
        min_d2=None,
        mean_exec_time_ns=res.mean_exec_time_ns,
        trace=res.instructions_and_trace[1] if res.instructions_and_trace else None,
        tmpdir=kwargs.get("tmpdir"),
    )

    allstats = np.stack([res.results[c]["stats"] for c in range(N_CORES)])
    f32 = np.float32
    neg_a = f32(allstats[:, :, 0].sum(dtype=np.float32))
    neg_b = f32(allstats[:, :, 1].sum(dtype=np.float32))
    negsum = f32(dbl * neg_a + neg_b)
    possum = f32(dbl * f32(allstats[:, :, 2].sum(dtype=np.float32)))
    min_d2 = f32(allstats[:, :, 3].min())

    LAST_RUN_INFO["min_d2"] = float(min_d2)
    positive_loss = f32(possum / f32(max(n_pos, 1)))
    negative_loss = f32(negsum / f32(max(n_neg, 1)))
    contrastive = f32(positive_loss - negative_loss + f32(MARGIN))

    d2r = f32(max(min_d2, f32(0.0)))
    d2r = f32(max(d2r, f32(1e-12)))
    min_inter = f32(np.sqrt(d2r))
    delta = f32(max(min_inter, f32(0.1)))
    constraint = f32(-C_PARAM / (delta * delta))
    reg = f32(LAMBDA_CURV * max(f32(0.0), f32(kappa_param - constraint)))
    out = f32(contrastive + reg)
    return np.asarray(out, dtype=np.float32)


# revision 17
# speedup vs baseline: 1.1263x; 1.0056x over previous
"""Trainium2 Bass kernel for nn_CurvatureAwareLoss.

Strategy (8 NeuronCores, symmetric data-parallel over batch rows):
  host:   sort rows by label. Core c owns rows [512c, 512c+512) and the
          upper-triangle-ish band of columns: global 512-col chunks
          {c..c+4} (mod 8), gathered into a packed [256, 2560] featT_c.
          Chunks 0..3 are doubled on the host (matrix symmetry); chunk 4
          (the antipodal band) is computed by both paired cores, counted 1x.
          Label-equality + triangle masks precomputed as small uint8 inputs.
  device: G = gcorr + X^T X / B  (PE, over full batch from featrows)
          FGT = G @ featT_c; s_j row via ones-matmul; fgt2 = -2*FGT[:,loc];
          d2 tiles = (-2 X_loc G X^T + s_j) via 3-matmul PSUM accumulation;
          s_i via ACT bias; masked exp sums via ACT accum_out;
          min via DVE reduce_min.
  host:   combine per-core partial sums/mins -> final f32 scalar.
"""

import contextlib
import ctypes
import json
import os
import sys
import tempfile
import types

import ml_dtypes
import numpy as np

N_CORES = 8
B = 4096
D = 256
RPC = B // N_CORES  # rows per core = 512
MCH = RPC // 128  # m-chunks per core = 4
JW = 512  # column chunk width
MARGIN = 1.0
LAMBDA_CURV = 0.1
C_PARAM = 1.0
C0 = float(2**23)  # mask displacement

LAST_RUN_INFO = {}

# ----------------------------------------------------------------------------
# harness shims
# ----------------------------------------------------------------------------


def _split_waits_json(bir: dict) -> dict:
    n = [0]
    for f in bir.get("functions", []):
        for blk in f.get("blocks", []):
            out = []
            for inst in blk.get("instructions", []):
                si = inst.get("sync_info") or {}
                waits = si.get("on_wait") or []
                if len(waits) > 1:
                    for w in waits[1:]:
                        n[0] += 1
                        out.append(
                            {
                                "debug": inst.get("debug", 0),
                                "engine": inst["engine"],
                                "ins": [],
                                "name": f"I-waitsplit-{n[0]}",
                                "opcode": "NoOp",
                                "outs": [],
                                "sync_info": {"on_update": [], "on_wait": [w]},
                            }
                        )
                    si = dict(si)
                    si["on_wait"] = waits[:1]
                    inst = dict(inst)
                    inst["sync_info"] = si
                out.append(inst)
            blk["instructions"] = out
    return bir


def _patch_nc(nc):
    orig = nc.to_json_bytes

    def patched():
        return json.dumps(_split_waits_json(json.loads(orig()))).encode()

    nc.to_json_bytes = patched
    return nc


def _install_ntff_hook():
    if "antenv.axon_hooks" in sys.modules:
        return
    so_path = "/opt/axon/libaxon_pjrt.so"
    hook = None
    try:
        lib = ctypes.CDLL(so_path)
        if hasattr(lib, "axon_start_nrt_profile"):
            lib.axon_start_nrt_profile.argtypes = [
                ctypes.POINTER(ctypes.c_int64),
                ctypes.c_size_t,
            ]
            lib.axon_start_nrt_profile.restype = ctypes.c_int64
            lib.axon_stop_nrt_profile.argtypes = [ctypes.c_char_p]
            lib.axon_stop_nrt_profile.restype = ctypes.c_int64

            @contextlib.contextmanager
            def _hook(output_dir, device_ids):
                import jax

                jax.devices()
                if device_ids:
                    ids = (ctypes.c_int64 * len(device_ids))(*device_ids)
                    rc = lib.axon_start_nrt_profile(ids, len(device_ids))
                else:
                    rc = lib.axon_start_nrt_profile(None, 0)
                if rc != 0:
                    raise RuntimeError(f"axon_start_nrt_profile rc={rc}")
                try:
                    yield
                finally:
                    k = lib.axon_stop_nrt_profile(str(output_dir).encode())
                    print(f"profile: {k} file(s) -> {output_dir}", file=sys.stderr)

            hook = _hook
    except OSError:
        hook = None
    mod = types.ModuleType("antenv.axon_hooks")
    mod.get_axon_ntff_profile_hook = lambda: hook
    mod.set_axon_ntff_profile_hook = lambda h: None
    sys.modules["antenv.axon_hooks"] = mod


# ----------------------------------------------------------------------------
# device program
# ----------------------------------------------------------------------------


def _build_program(a_scale: float, nj: int, masked_js, mask_dt_np, pos_windows=None):
    """a_scale = -kappa > 0. nj: number of 512-col program chunks per core.
    masked_js: j chunks with label/triangle masks. mask_dt_np: mask np dtype."""
    from contextlib import ExitStack

    import concourse.bass as bass
    import concourse.tile as tile
    from concourse import mybir

    f32 = mybir.dt.float32
    f32r = mybir.dt.float32r
    mdt = mybir.dt.from_np(np.dtype(mask_dt_np))
    AF = mybir.ActivationFunctionType
    OP = mybir.AluOpType
    X = mybir.AxisListType.X

    a = float(a_scale)
    kappa = -a
    W = nj * JW  # program column width
    mw = len(masked_js) * JW  # mask width

    nc = bass.Bass("TRN2", target_bir_lowering=False, debug=False,
                   num_devices=N_CORES)

    featT = nc.dram_tensor("featT", [D, W], f32r, kind="ExternalInput").ap()
    featrows = nc.dram_tensor("featrows", [B, D], f32r, kind="ExternalInput").ap()
    amask = nc.dram_tensor("amask", [MCH, 128, mw], mdt, kind="ExternalInput").ap()
    bmask = nc.dram_tensor("bmask", [MCH, 128, mw], mdt, kind="ExternalInput").ap()
    gcorr = nc.dram_tensor("gcorr", [D, D], f32, kind="ExternalInput").ap()
    stats = nc.dram_tensor("stats", [128, 4], f32, kind="ExternalOutput").ap()
    sbounce = nc.dram_tensor("sbounce", [RPC], f32, kind="Internal").ap()

    with tile.TileContext(nc) as tc, ExitStack() as ctx:
        consts = ctx.enter_context(tc.tile_pool(name="consts", bufs=1))

        featT_sb = [consts.tile([128, W], f32r, name=f"featT{k}", tag=f"featT{k}")
                    for k in range(2)]
        am_sb = [consts.tile([128, mw], mdt, name=f"am{m}", tag=f"am{m}")
                 for m in range(MCH)]
        bm_sb = [consts.tile([128, mw], mdt, name=f"bm{m}", tag=f"bm{m}")
                 for m in range(MCH)]
        gcorr_sb = [consts.tile([128, D], f32, name=f"gc{k}", tag=f"gc{k}")
                    for k in range(2)]
        g_sb = [consts.tile([128, D], f32r, name=f"g{k}", tag=f"g{k}")
                for k in range(2)]
        fgt2 = [consts.tile([128, JW], f32r, name=f"fgt2{k}", tag=f"fgt2{k}")
                for k in range(2)]
        augR = consts.tile([1, W], f32r, name="augR", tag="augR")
        onescol = consts.tile([128, 1], f32r, name="onescol", tag="onescol")
        onesrow = consts.tile([1, 128], f32r, name="onesrow", tag="onesrow")
        scol4 = consts.tile([128, MCH], f32, name="scol4", tag="scol4")
        biasneg = consts.tile([128, MCH], f32, name="biasneg", tag="biasneg")
        biaspos = consts.tile([128, MCH], f32, name="biaspos", tag="biaspos")
        negacc = [consts.tile([128, nj], f32, name=f"negacc{m}", tag=f"negacc{m}")
                  for m in range(MCH)]
        posacc = [consts.tile([128, nj], f32, name=f"posacc{m}", tag=f"posacc{m}")
                  for m in range(MCH)]
        minacc = [consts.tile([128, nj], f32, name=f"minacc{m}", tag=f"minacc{m}")
                  for m in range(MCH)]
        stats_sb = consts.tile([128, 4], f32, name="stats_sb", tag="stats_sb")

        # PE warmup tile
        warm = consts.tile([128, JW], f32r, name="warm", tag="warm")
        nc.vector.memset(warm[:].bitcast(f32), 0.5)
        wdump = consts.tile([128, 1], f32, name="wdump", tag="wdump")

        # ---------------- DMAs: gcorr, frs, featT j01, masks, featT rest --
        for k in range(2):
            nc.gpsimd.dma_start(out=gcorr_sb[k][:],
                                in_=gcorr[k * 128:(k + 1) * 128, :])
        frs_ctx = tc.tile_pool(name="frs", bufs=4 if nj == 5 else 2)
        frs_pool = frs_ctx.__enter__()
        fr_r = featrows.rearrange("(g p) d -> g p d", p=128)  # [32,128,256]
        frs_tiles = []
        for blk in range(4):
            frs = frs_pool.tile([128, 8, D], f32r, name=f"frs{blk}",
                                tag=f"frs{blk}")
            nc.gpsimd.dma_start(
                out=frs[:],
                in_=fr_r[blk * 8:(blk + 1) * 8, :, :].rearrange("g p d -> p g d"),
            )
            frs_tiles.append(frs)
        for j in range(2):
            for k in range(2):
                nc.gpsimd.dma_start(
                    out=featT_sb[k][:, j * JW:(j + 1) * JW],
                    in_=featT[k * 128:(k + 1) * 128, j * JW:(j + 1) * JW],
                )
        for m in range(MCH):
            nc.gpsimd.dma_start(out=am_sb[m][:], in_=amask[m, :, :])
            nc.gpsimd.dma_start(out=bm_sb[m][:], in_=bmask[m, :, :])
        for j in range(2, nj):
            for k in range(2):
                nc.gpsimd.dma_start(
                    out=featT_sb[k][:, j * JW:(j + 1) * JW],
                    in_=featT[k * 128:(k + 1) * 128, j * JW:(j + 1) * JW],
                )

        nc.vector.memset(onescol[:].bitcast(f32), 1.0)
        nc.vector.memset(onesrow[:].bitcast(f32), 1.0)
        for m in range(MCH):
            nc.vector.memset(posacc[m][:], 0.0)

        # PE warmup while DMAs stream
        with tc.tile_pool(name="wps", bufs=1, space="PSUM") as wps_pool:
            wps = wps_pool.tile([128, JW], f32, name="wps", tag="wps")
            for w in range(20):
                nc.tensor.matmul(wps[:], lhsT=warm[:, 0:128], rhs=warm[:],
                                 start=(w == 0), stop=(w == 19),
                                 skip_group_check=True)
            nc.vector.tensor_reduce(out=wdump[:], in_=wps[:, 0:1],
                                    axis=X, op=OP.max)
        with tc.tile_pool(name="gps", bufs=2, space="PSUM") as gps_pool:
            g_ps = [gps_pool.tile([128, D], f32, name=f"gps{k}", tag=f"gps{k}")
                    for k in range(2)]
            for blk in range(4):
                frs = frs_tiles[blk]
                for g in range(8):
                    t = blk * 8 + g
                    xb = frs[:, g, :]
                    for kc in range(2):
                        nc.tensor.matmul(
                            g_ps[kc][:],
                            lhsT=xb[:, kc * 128:(kc + 1) * 128],
                            rhs=xb,
                            start=(t == 0),
                            stop=(t == 31),
                            skip_group_check=True,
                        )
            for kc in range(2):
                nc.vector.scalar_tensor_tensor(
                    out=g_sb[kc][:],
                    in0=g_ps[kc][:],
                    scalar=1.0 / B,
                    in1=gcorr_sb[kc][:],
                    op0=OP.mult,
                    op1=OP.add,
                )
        frs_ctx.__exit__(None, None, None)

        # ---------------- FGT, s-row, fgt2, s_col -------------------------
        prod_pool = ctx.enter_context(tc.tile_pool(name="prod", bufs=4))
        with tc.tile_pool(name="fps", bufs=3, space="PSUM") as fps_pool, \
             tc.tile_pool(name="sps", bufs=3, space="PSUM") as sps_pool:
            for j in range(nj):
                jsl = slice(j * JW, (j + 1) * JW)
                s_ps = sps_pool.tile([1, JW], f32, name="s_ps", tag="s_ps")
                for kc in range(2):
                    f_ps = fps_pool.tile([128, JW], f32, name="f_ps", tag="f_ps")
                    for ki in range(2):
                        nc.tensor.matmul(
                            f_ps[:],
                            lhsT=g_sb[ki][:, kc * 128:(kc + 1) * 128],
                            rhs=featT_sb[ki][:, jsl],
                            start=(ki == 0),
                            stop=(ki == 1),
                            skip_group_check=True,
                        )
                    if j == 0:
                        # local cols are program chunk 0
                        nc.scalar.mul(out=fgt2[kc][:],
                                      in_=f_ps[:], mul=-2.0)
                    prod = prod_pool.tile([128, JW], f32r, name="prod",
                                          tag="prod")
                    nc.vector.scalar_tensor_tensor(
                        out=prod[:], in0=f_ps[:], scalar=1.0,
                        in1=featT_sb[kc][:, jsl].bitcast(f32),
                        op0=OP.mult, op1=OP.mult)
                    nc.tensor.matmul(
                        s_ps[:],
                        lhsT=onescol[:],
                        rhs=prod[:],
                        start=(kc == 0),
                        stop=(kc == 1),
                        skip_group_check=True,
                    )
                nc.scalar.copy(out=augR[0:1, jsl], in_=s_ps[:])
                if j == 0:
                    nc.gpsimd.dma_start(out=sbounce[:],
                                        in_=augR[0:1, 0:RPC].bitcast(f32))
                    nc.gpsimd.dma_start(
                        out=scol4[:],
                        in_=bass.AP(sbounce.tensor, 0, [[1, 128], [128, MCH]]),
                    )
                    nc.vector.tensor_scalar_mul(out=biasneg[:], in0=scol4[:],
                                                scalar1=a)
                    nc.vector.tensor_scalar_mul(out=biaspos[:], in0=scol4[:],
                                                scalar1=kappa)

        # ---------------- main pairwise tiles -----------------------------
        scr_pool = ctx.enter_context(tc.tile_pool(name="scr", bufs=3))
        dump_pool = ctx.enter_context(tc.tile_pool(name="dump", bufs=4))
        masked_pairs = [(m, j) for j in masked_js for m in range(MCH)]
        clean_pairs = [(m, j) for j in range(nj) if j not in masked_js
                       for m in range(MCH)]
        tile_order = []
        ci = 0
        for mp in masked_pairs:
            tile_order.append(mp)
            if ci < len(clean_pairs):
                tile_order.append(clean_pairs[ci])
                ci += 1
        tile_order.extend(clean_pairs[ci:])
        with tc.tile_pool(name="mps", bufs=8, space="PSUM") as mps_pool:
            for m, j in tile_order:
                jsl = slice(j * JW, (j + 1) * JW)
                if True:
                    ps = mps_pool.tile([128, JW], f32, name="ps", tag="ps")
                    for ki in range(2):
                        nc.tensor.matmul(
                            ps[:],
                            lhsT=fgt2[ki][:, m * 128:(m + 1) * 128],
                            rhs=featT_sb[ki][:, jsl],
                            start=(ki == 0),
                            stop=False,
                            skip_group_check=True,
                        )
                    nc.tensor.matmul(
                        ps[:],
                        lhsT=onesrow[:],
                        rhs=augR[0:1, jsl],
                        start=False,
                        stop=True,
                        skip_group_check=True,
                    )
                    bneg = biasneg[:, m:m + 1]
                    bpos = biaspos[:, m:m + 1]
                    if j in masked_js:
                        jj = masked_js.index(j)
                        at = am_sb[m][:, jj * JW:(jj + 1) * JW]
                        bt = bm_sb[m][:, jj * JW:(jj + 1) * JW]
                        q = scr_pool.tile([128, JW], f32, name="q", tag="q")
                        nc.vector.scalar_tensor_tensor(
                            out=q[:], in0=at, scalar=C0, in1=ps[:],
                            op0=OP.mult, op1=OP.subtract)
                        tmin = scr_pool.tile([128, JW], f32, name="tmin",
                                             tag="tmin")
                        nc.vector.scalar_tensor_tensor(
                            out=tmin[:], in0=at, scalar=C0, in1=ps[:],
                            op0=OP.mult, op1=OP.add)
                        win = (pos_windows or {}).get((m, j), (0, JW))
                        dump = dump_pool.tile([128, JW], f32, name="dump",
                                              tag="dump")
                        nc.scalar.activation(
                            out=dump[:], in_=q[:], func=AF.Exp, scale=kappa,
                            bias=bneg, accum_out=negacc[m][:, j:j + 1])
                        if win is not None:
                            lo, hi = win
                            wsl = slice(lo, hi)
                            tpos = scr_pool.tile([128, JW], f32, name="tpos",
                                                 tag="tpos")
                            nc.vector.scalar_tensor_tensor(
                                out=tpos[:, wsl],
                                in0=bt[:, wsl], scalar=C0, in1=ps[:, wsl],
                                op0=OP.mult, op1=OP.add)
                            dump2 = dump_pool.tile([128, JW], f32, name="dump",
                                                   tag="dump")
                            nc.scalar.activation(
                                out=dump2[:, wsl], in_=tpos[:, wsl],
                                func=AF.Exp, scale=kappa, bias=bpos,
                                accum_out=posacc[m][:, j:j + 1])
                        nc.vector.tensor_reduce(
                            out=minacc[m][:, j:j + 1], in_=tmin[:], axis=X,
                            op=OP.min)
                    else:
                        dump = dump_pool.tile([128, JW], f32, name="dump",
                                              tag="dump")
                        nc.scalar.activation(
                            out=dump[:], in_=ps[:], func=AF.Exp, scale=a,
                            bias=bneg, accum_out=negacc[m][:, j:j + 1])
                        nc.vector.tensor_reduce(
                            out=minacc[m][:, j:j + 1], in_=ps[:], axis=X,
                            op=OP.min)

        # ---------------- finalize ----------------------------------------
        # stats: col0 = neg sum j<nj-1 (doubled on host), col1 = neg j=nj-1,
        #        col2 = pos sum, col3 = min d2
        fin = ctx.enter_context(tc.tile_pool(name="fin", bufs=1))
        negrow = fin.tile([128, MCH], f32, name="negrow", tag="negrow")
        negrow4 = fin.tile([128, MCH], f32, name="negrow4", tag="negrow4")
        posrow = fin.tile([128, MCH], f32, name="posrow", tag="posrow")
        minrow = fin.tile([128, MCH], f32, name="minrow", tag="minrow")
        for m in range(MCH):
            nc.vector.tensor_reduce(out=negrow[:, m:m + 1],
                                    in_=negacc[m][:, 0:nj - 1], axis=X,
                                    op=OP.add)
            nc.vector.tensor_copy(out=negrow4[:, m:m + 1],
                                  in_=negacc[m][:, nj - 1:nj])
            nc.vector.tensor_reduce(out=posrow[:, m:m + 1], in_=posacc[m][:],
                                    axis=X, op=OP.add)
            rm = fin.tile([128, 1], f32, name=f"rm{m}", tag=f"rm{m}")
            nc.vector.tensor_reduce(out=rm[:], in_=minacc[m][:], axis=X,
                                    op=OP.min)
            nc.vector.tensor_add(out=minrow[:, m:m + 1], in0=rm[:],
                                 in1=scol4[:, m:m + 1])
        nc.vector.tensor_reduce(out=stats_sb[:, 0:1], in_=negrow[:], axis=X,
                                op=OP.add)
        nc.vector.tensor_reduce(out=stats_sb[:, 1:2], in_=negrow4[:], axis=X,
                                op=OP.add)
        nc.vector.tensor_reduce(out=stats_sb[:, 2:3], in_=posrow[:], axis=X,
                                op=OP.add)
        nc.vector.tensor_reduce(out=stats_sb[:, 3:4], in_=minrow[:], axis=X,
                                op=OP.min)
        nc.sync.dma_start(out=stats, in_=stats_sb[:])

    return _patch_nc(nc)


# ----------------------------------------------------------------------------
# host wrapper
# ----------------------------------------------------------------------------


def kernel(features, w1, b1, w2, b2, kappa_param, labels):
    features = np.asarray(features, dtype=np.float32)
    w1 = np.asarray(w1, dtype=np.float32)
    b1 = np.asarray(b1, dtype=np.float32)
    w2 = np.asarray(w2, dtype=np.float32)
    b2 = np.asarray(b2, dtype=np.float32)
    kappa_param = np.float32(np.asarray(kappa_param))
    labels_i = np.asarray(labels).astype(np.int64)

    assert features.shape == (B, D)

    # ---- tiny MLP -> kappa ----
    mu = features.mean(axis=0, dtype=np.float32).astype(np.float32)
    h = np.tanh(mu @ w1 + b1).astype(np.float32)
    z = np.float32((h @ w2 + b2)[0])
    softplus = np.float32(np.logaddexp(np.float32(0.0), z))
    kappa = np.float32(-softplus)
    a = float(-kappa)

    _, counts = np.unique(labels_i, return_counts=True)
    n_pos = int((counts * (counts - 1)).sum())
    n_neg = int(B * (B - 1) - n_pos)

    order = np.argsort(labels_i, kind="stable")
    Xs = np.ascontiguousarray(features[order])
    ls = labels_i[order]
    featT_s = np.ascontiguousarray(Xs.T)  # [D, B]

    gcorr = (np.eye(D, dtype=np.float32) - np.outer(mu, mu)).astype(np.float32)
    mask_np = np.uint8

    # triangle exclusion (chunk 0): excl iff q <= 128m + p
    qs = np.arange(JW)
    tri = np.zeros((MCH, 128, JW), dtype=np.int32)
    for m in range(MCH):
        for p in range(128):
            tri[m, p] = (qs <= 128 * m + p)

    # symmetric mode: core c -> global chunks {c..c+4}; masked {0,1}.
    # Same-label cells of core c's rows must lie in global cols
    # [512c - 512, 512c + 1024): below-diagonal part is triangle/mirror
    # covered, the rest must fall in the two masked chunks.
    nj = 5
    masked_js = [0, 1]
    sym_ok = True
    for c in range(N_CORES):
        rows = ls[c * RPC:(c + 1) * RPC]
        inside = np.zeros(B, dtype=bool)
        for g in range(c * RPC - RPC, c * RPC + 1024):
            inside[g % B] = True
        if (rows[:, None] == ls[None, inside == False]).any():  # noqa: E712
            sym_ok = False
            break

    if not sym_ok:
        nj = N_CORES
        masked_js = list(range(nj))
    mw = len(masked_js) * JW
    dbl = np.float32(2.0) if sym_ok else np.float32(1.0)

    in_maps = []
    for c in range(N_CORES):
        gchunks = [(c + t) % N_CORES for t in range(nj)]
        cols = np.concatenate([np.arange(g * JW, (g + 1) * JW)
                               for g in gchunks])
        featT_c = np.ascontiguousarray(featT_s[:, cols])
        rows = ls[c * RPC:(c + 1) * RPC]
        lcols = ls[cols[:mw]]
        am = np.zeros((MCH, 128, mw), dtype=np.int32)
        bm = np.zeros((MCH, 128, mw), dtype=np.int32)
        for m in range(MCH):
            r = rows[m * 128:(m + 1) * 128]
            eq = (r[:, None] == lcols[None, :]).astype(np.int32)
            am[m] = eq
            bm[m] = 1 - eq
            if sym_ok:
                am[m, :, :JW] += tri[m]
                bm[m, :, :JW] += tri[m]
            else:
                # exclude the diagonal from pos explicitly
                for p in range(128):
                    g = c * RPC + m * 128 + p
                    pos = int(np.where(cols == g)[0][0])
                    bm[m, p, pos] += 1
        in_maps.append(
            {
                "featT": featT_c,
                "featrows": Xs,
                "amask": am.astype(mask_np),
                "bmask": bm.astype(mask_np),
                "gcorr": gcorr,
            }
        )

    pos_windows = None
    if sym_ok and int(counts.max()) <= 300:
        # kept pos cells of m-chunk m lie in prog cols [128m, 128m+448)
        pos_windows = {}
        for m in range(MCH):
            lo_g, hi_g = 128 * m, 128 * m + 448
            for j in masked_js:
                lo = max(lo_g - j * JW, 0)
                hi = min(hi_g - j * JW, JW)
                pos_windows[(m, j)] = (lo, hi) if hi > lo else None

    trace = bool(os.environ.get("BASS_TRACE"))
    if trace:
        _install_ntff_hook()
    nc = _build_program(a, nj, masked_js, mask_np, pos_windows)

    from concourse.bass_utils import run_bass_kernel_spmd

    kwargs = {}
    if trace:
        kwargs["tmpdir"] = tempfile.mkdtemp(prefix="curvloss_trace_")
    res = run_bass_kernel_spmd(nc, in_maps, core_ids=list(range(N_CORES)),
                               trace=trace, **kwargs)
    LAST_RUN_INFO.clear()
    LAST_RUN_INFO.update(
        exec_time_ns=res.exec_time_ns,
        min_d2=None,
        mean_exec_time_ns=res.mean_exec_time_ns,
        trace=res.instructions_and_trace[1] if res.instructions_and_trace else None,
        tmpdir=kwargs.get("tmpdir"),
    )

    allstats = np.stack([res.results[c]["stats"] for c in range(N_CORES)])
    f32 = np.float32
    neg_a = f32(allstats[:, :, 0].sum(dtype=np.float32))
    neg_b = f32(allstats[:, :, 1].sum(dtype=np.float32))
    negsum = f32(dbl * neg_a + neg_b)
    possum = f32(dbl * f32(allstats[:, :, 2].sum(dtype=np.float32)))
    min_d2 = f32(allstats[:, :, 3].min())

    LAST_RUN_INFO["min_d2"] = float(min_d2)
    positive_loss = f32(possum / f32(max(n_pos, 1)))
    negative_loss = f32(negsum / f32(max(n_neg, 1)))
    contrastive = f32(positive_loss - negative_loss + f32(MARGIN))

    d2r = f32(max(min_d2, f32(0.0)))
    d2r = f32(max(d2r, f32(1e-12)))
    min_inter = f32(np.sqrt(d2r))
    delta = f32(max(min_inter, f32(0.1)))
    constraint = f32(-C_PARAM / (delta * delta))
    reg = f32(LAMBDA_CURV * max(f32(0.0), f32(kappa_param - constraint)))
    out = f32(contrastive + reg)
    return np.asarray(out, dtype=np.float32)
